# revision 9
# baseline (speedup 1.0000x reference)
"""Trainium2 Bass kernel for nn_CrossAttentionLayer (B=4, N=1024, M=4096,
DQ=DC=1024, H=16, DH=64).

Sharding: 8 cores = 4 batches x 2 half-head-groups. Core c handles batch
c//2 and heads [8*(c%2), 8*(c%2)+8). Each core computes its partial
out = concat_heads(attn) @ Wo_slice; host sums the two partials per batch
and adds the bias (the tensor-parallel all-reduce done host-side).

Math identity used on device:
  P = clamp(exp(scale*S + madd), e^-5, e^5)   with madd = 0 / -1000 (mask)
is exactly exp(clip(where(mask, scale*S, -inf), -5, 5)) since exp is
monotone and exp(-1000+s) underflows to 0 -> clamps to e^-5.
The softmax denominator is obtained by appending a ones column to V in the
P@V matmul (row 64 of the [65, n] output).

All matmuls run as float32r (TF32-like, full PE rate at free-dim>=256).
S^T matmuls (contraction d=64) use 2x row tiling (tile_position (0,0) /
(64,0)) to run both heads of a pair concurrently on the PE array.

Schedule: head 0's attention is interleaved into the K/V projection phase
(PE-bound there, ScalarE idle) so its exp cost is hidden; head 1 and pairs
1-3 run after, ScalarE(exp)-bound.
"""
import sys
sys.path.insert(0, '/opt/trn_rl_repo')
from contextlib import ExitStack

import numpy as np

import concourse.bass as bass  # noqa: F401
import concourse.mybir as mybir
import concourse.tile as tile
from concourse import bacc
from concourse.bass_utils import run_bass_kernel_spmd
from concourse.masks import make_identity

F32 = mybir.dt.float32
F32R = mybir.dt.float32r
AF = mybir.ActivationFunctionType
ALU = mybir.AluOpType

B, N, M = 4, 1024, 4096
DQ = 1024
NHC = 8              # heads per core
D = 64
IC = NHC * D         # 512 inner dims per core
NP = NHC // 2        # 4 head pairs per core
MC = M // 128        # 32 context chunks of 128
E5 = float(np.exp(np.float32(5.0)))
EM5 = float(np.exp(np.float32(-5.0)))
SCALE = float(D) ** -0.5  # 0.125

_CACHE = {}


def _emit(nc, tc, tensors):
    x_d, ctx_d, wq_d, wk_d, wv_d, wo_d, madd_d, out_d = tensors

    with nc.allow_low_precision(reason="fp32r matmul operands"), ExitStack() as ctx:
        persist = ctx.enter_context(tc.tile_pool(name="persist", bufs=1))
        dram = ctx.enter_context(tc.tile_pool(name="dram", bufs=1, space="DRAM"))

        madd_sb = persist.tile([128, MC], F32, tag="madd")
        nc.sync.dma_start(madd_sb[:], madd_d[:])
        ident = persist.tile([128, 128], F32, tag="ident")
        make_identity(nc, ident[:])
        ones_f = persist.tile([128, 1], F32, tag="onesf")
        nc.vector.memset(ones_f[:], 1.0)
        ones_r = persist.tile([1, 64], F32R, tag="onesr")
        nc.vector.tensor_copy(ones_r[:], ones_f[0:1, 0:1].to_broadcast((1, 64)))

        QT = [persist.tile([128, N], F32R, tag=f"qt{p}", name=f"qt{p}")
              for p in range(NP)]
        V = [persist.tile([128, NHC * 65], F32R, tag=f"v{mc}", name=f"v{mc}")
             for mc in range(MC)]
        OnT = [persist.tile([128, N], F32R, tag=f"ont{p}", name=f"ont{p}")
               for p in range(NP)]
        ktd = dram.tile([NP, 8, 128, 512], F32R)

        ecnt = [0]

        def evac(out_ap, in_ap):
            # alternate PSUM evacuations between ScalarE and VectorE
            # (Exp/Copy share one ACT table set, so no table-switch cost)
            if ecnt[0] % 2 == 0:
                nc.scalar.copy(out_ap, in_ap)
            else:
                nc.vector.tensor_copy(out_ap, in_ap)
            ecnt[0] += 1

        def norm_head(O2, p, h2, prb, psR, rtag):
            """Normalize one head's [65, 512] O tiles into OnT[p] rows."""
            for nh in range(2):
                rc = prb.tile([1, 512], F32R, tag="rc",
                              name=f"rc{p}{h2}{nh}")
                nc.vector.reciprocal(rc[:], O2[nh][64:65, :])
                Rb = psR.tile([64, 512], F32, tag=rtag,
                              name=f"rb{p}{h2}{nh}")
                nc.tensor.matmul(Rb[:], ones_r[:], rc[:],
                                 start=True, stop=True)
                rbs = prb.tile([64, 512], F32, tag="rbs",
                               name=f"rbs{p}{h2}{nh}")
                nc.vector.tensor_copy(rbs[:], Rb[:])
                nc.vector.tensor_tensor(
                    OnT[p][h2 * 64:(h2 + 1) * 64,
                           nh * 512:(nh + 1) * 512],
                    O2[nh][0:64, :], rbs[:], ALU.mult)

        # ---- Phase A/B: x -> x^T, Q^T = (x @ Wq)^T ----
        with tc.tile_pool(name="pa", bufs=1) as pa, \
             tc.tile_pool(name="psA", bufs=2, space="PSUM") as psA:
            wq_r = pa.tile([128, 8, IC], F32R, tag="wq")
            nc.gpsimd.dma_start(wq_r[:], wq_d.rearrange("(c p) i -> p c i", p=128))
            x_ld = pa.tile([128, 8, DQ], F32, tag="xld")
            for q in range(4):
                nc.sync.dma_start(
                    x_ld[:, 2 * q:2 * q + 2, :],
                    x_d[q * 256:(q + 1) * 256, :].rearrange(
                        "(s p) d -> p s d", p=128))
            xT = pa.tile([128, 8, N], F32R, tag="xt")
            for nh in range(2):
                for dc in range(8):
                    pt = psA.tile([128, 512], F32, tag="tp")
                    for s in range(4):
                        nc.tensor.transpose(
                            pt[:, s * 128:(s + 1) * 128],
                            x_ld[:, nh * 4 + s, dc * 128:(dc + 1) * 128],
                            ident[:])
                    evac(xT[:, dc, nh * 512:(nh + 1) * 512], pt[:])
                for p in range(NP):
                    qp = psA.tile([128, 512], F32, tag="qp")
                    for dc in range(8):
                        nc.tensor.matmul(
                            qp[:], wq_r[:, dc, p * 128:(p + 1) * 128],
                            xT[:, dc, nh * 512:(nh + 1) * 512],
                            start=(dc == 0), stop=(dc == 7))
                    evac(QT[p][:, nh * 512:(nh + 1) * 512], qp[:])

        # ---- Phase C: ctx^T, K^T (->DRAM), V + interleaved head-0 attn ----
        with tc.tile_pool(name="pc", bufs=1) as pc, \
             tc.tile_pool(name="pcl", bufs=2) as pcl, \
             tc.tile_pool(name="pks", bufs=4) as pks, \
             tc.tile_pool(name="pp0", bufs=3) as pp0, \
             tc.tile_pool(name="prb0", bufs=2) as prb0, \
             tc.tile_pool(name="psT", bufs=2, space="PSUM") as psT, \
             tc.tile_pool(name="psKV", bufs=2, space="PSUM") as psKV, \
             tc.tile_pool(name="psS0", bufs=2, space="PSUM") as psS0, \
             tc.tile_pool(name="psO0", bufs=1, space="PSUM") as psO0:
            O0 = [psO0.tile([65, 512], F32, tag=f"o0_{nh}", name=f"o0_{nh}")
                  for nh in range(2)]
            wk_r = pc.tile([128, 8, IC], F32R, tag="wk")
            nc.gpsimd.dma_start(wk_r[:], wk_d.rearrange("(c p) i -> p c i", p=128))
            wv_r = pc.tile([128, 8, IC], F32R, tag="wv")
            nc.gpsimd.dma_start(wv_r[:], wv_d.rearrange("(c p) i -> p c i", p=128))
            for m5 in range(8):
                ctx_ld = pcl.tile([128, 2, DQ], F32, tag="cld",
                                  name=f"cld{m5}a")
                nc.sync.dma_start(
                    ctx_ld[:],
                    ctx_d[m5 * 512:m5 * 512 + 256, :].rearrange(
                        "(s p) d -> p s d", p=128))
                ctx_ld2 = pcl.tile([128, 2, DQ], F32, tag="cld",
                                   name=f"cld{m5}b")
                nc.sync.dma_start(
                    ctx_ld2[:],
                    ctx_d[m5 * 512 + 256:(m5 + 1) * 512, :].rearrange(
                        "(s p) d -> p s d", p=128))
                lds = (ctx_ld, ctx_ld, ctx_ld2, ctx_ld2)
                ctxT = pcl.tile([128, 8, 512], F32R, tag="ctxT",
                                name=f"ctxT{m5}")
                for dc in range(8):
                    pt = psT.tile([128, 512], F32, tag="tp", name=f"pt{m5}_{dc}")
                    for s in range(4):
                        nc.tensor.transpose(
                            pt[:, s * 128:(s + 1) * 128],
                            lds[s][:, s % 2, dc * 128:(dc + 1) * 128], ident[:])
                    evac(ctxT[:, dc, :], pt[:])
                ks0 = None
                for p in range(NP):
                    kp = psKV.tile([128, 512], F32, tag="kv", name=f"kp{m5}_{p}")
                    for dc in range(8):
                        nc.tensor.matmul(
                            kp[:], wk_r[:, dc, p * 128:(p + 1) * 128],
                            ctxT[:, dc, :], start=(dc == 0), stop=(dc == 7))
                    ks = pks.tile([128, 512], F32R, tag="ks", name=f"ks{m5}_{p}")
                    evac(ks[:], kp[:])
                    nc.sync.dma_start(ktd[p, m5], ks[:])
                    if p == 0:
                        ks0 = ks
                for s in range(4):
                    vp = psKV.tile([128, 512], F32, tag="kv", name=f"vp{m5}_{s}")
                    for dc in range(8):
                        nc.tensor.matmul(
                            vp[:], ctxT[:, dc, s * 128:(s + 1) * 128],
                            wv_r[:, dc, :], start=(dc == 0), stop=(dc == 7))
                    mc = m5 * 4 + s
                    v3 = V[mc].rearrange("q (h e) -> q h e", e=65)
                    evac(v3[:, :, 0:64], vp[:].rearrange("q (h e) -> q h e", e=64))
                    nc.vector.tensor_copy(
                        v3[:, :, 64:65],
                        ones_f[:, 0:1, None].to_broadcast((128, NHC, 1)))
                # head-0 attention over this m5 block (hides exp under PE work)
                for s in range(4):
                    mc = m5 * 4 + s
                    for nh in range(2):
                        S = psS0.tile([128, 512], F32, tag="s0",
                                      name=f"s0_{mc}_{nh}")
                        nc.tensor.matmul(
                            S[:], ks0[0:64, s * 128:(s + 1) * 128],
                            QT[0][0:64, nh * 512:(nh + 1) * 512],
                            start=True, stop=True, tile_position=(0, 0))
                        P_sb = pp0.tile([128, 512], F32R, tag="p0",
                                        name=f"p0_{mc}_{nh}")
                        nc.scalar.activation(
                            P_sb[:], S[:], AF.Exp,
                            bias=madd_sb[:, mc:mc + 1], scale=SCALE)
                        nc.vector.tensor_scalar(
                            P_sb[:], P_sb[:], E5, EM5, ALU.min, ALU.max)
                        nc.tensor.matmul(
                            O0[nh][:], V[mc][:, 0:65], P_sb[:],
                            start=(mc == 0), stop=(mc == MC - 1))
            norm_head(O0, 0, 0, prb0, psS0, "s0")

        # ---- Attention: head 1 (solo), then pairs 1-3 ----
        pwo = ctx.enter_context(tc.tile_pool(name="pwo", bufs=1))
        wo_r = pwo.tile([128, NP, DQ], F32R, tag="wo")
        with tc.tile_pool(name="pkt", bufs=4) as pkt, \
             tc.tile_pool(name="pp", bufs=3) as pp, \
             tc.tile_pool(name="prb", bufs=2) as prb, \
             tc.tile_pool(name="psS", bufs=2, space="PSUM") as psS, \
             tc.tile_pool(name="psO", bufs=1, space="PSUM") as psO:
            nc.gpsimd.dma_start(wo_r[:], wo_d.rearrange("(p q) d -> q p d", q=128))
            # head 1: K^T rows 64-127 of pair 0, tile_position (64,0)
            O1 = [psO.tile([65, 512], F32, tag=f"oo0{nh}", name=f"o1_{nh}")
                  for nh in range(2)]
            for m5 in range(8):
                kts = pkt.tile([128, 512], F32R, tag="kts", name=f"kts0_{m5}")
                nc.sync.dma_start(kts[:], ktd[0, m5])
                for s in range(4):
                    mc = m5 * 4 + s
                    S = psS.tile([128, 1024], F32, tag="s",
                                 name=f"s1_{mc}")
                    for nh in range(2):
                        nc.tensor.matmul(
                            S[:, nh * 512:(nh + 1) * 512],
                            kts[64:128, s * 128:(s + 1) * 128],
                            QT[0][64:128, nh * 512:(nh + 1) * 512],
                            start=True, stop=True, tile_position=(64, 0))
                    P_sb = pp.tile([128, 1024], F32R, tag="p", name=f"p1_{mc}")
                    nc.scalar.activation(
                        P_sb[:], S[:], AF.Exp,
                        bias=madd_sb[:, mc:mc + 1], scale=SCALE)
                    nc.vector.tensor_scalar(
                        P_sb[:], P_sb[:], E5, EM5, ALU.min, ALU.max)
                    for nh in range(2):
                        nc.tensor.matmul(
                            O1[nh][:], V[mc][:, 65:130],
                            P_sb[:, nh * 512:(nh + 1) * 512],
                            start=(mc == 0), stop=(mc == MC - 1))
            norm_head(O1, 0, 1, prb, psS, "s")

            # pairs 1-3: one n-half per pass so O needs only 2 PSUM banks,
            # leaving S at bufs=3 (exp never starves on the slot rotation)
            for p in range(1, NP):
                for nh in range(2):
                    par = (2 * p + nh) % 2
                    O = [psO.tile([65, 512], F32, tag=f"oo{par}{h2}",
                                  name=f"o{p}_{h2}{nh}") for h2 in range(2)]
                    for m5 in range(8):
                        kts = pkt.tile([128, 512], F32R, tag="kts",
                                       name=f"kts{p}_{m5}_{nh}")
                        nc.sync.dma_start(kts[:], ktd[p, m5])
                        for s in range(4):
                            mc = m5 * 4 + s
                            S = psS.tile([128, 1024], F32, tag="s",
                                         name=f"s{p}_{mc}_{nh}")
                            nc.tensor.matmul(
                                S[:, 0:512],
                                kts[0:64, s * 128:(s + 1) * 128],
                                QT[p][0:64, nh * 512:(nh + 1) * 512],
                                start=True, stop=True, tile_position=(0, 0))
                            nc.tensor.matmul(
                                S[:, 512:1024],
                                kts[64:128, s * 128:(s + 1) * 128],
                                QT[p][64:128, nh * 512:(nh + 1) * 512],
                                start=True, stop=True, tile_position=(64, 0))
                            P_sb = pp.tile([128, 1024], F32R, tag="p",
                                           name=f"p{p}_{mc}_{nh}")
                            nc.scalar.activation(
                                P_sb[:], S[:], AF.Exp,
                                bias=madd_sb[:, mc:mc + 1], scale=SCALE)
                            nc.vector.tensor_scalar(
                                P_sb[:], P_sb[:], E5, EM5, ALU.min, ALU.max)
                            for h2 in range(2):
                                h = 2 * p + h2
                                nc.tensor.matmul(
                                    O[h2][:],
                                    V[mc][:, h * 65:(h + 1) * 65],
                                    P_sb[:, h2 * 512:(h2 + 1) * 512],
                                    start=(mc == 0), stop=(mc == MC - 1))
                    for h2 in range(2):
                        rc = prb.tile([1, 512], F32R, tag="rc",
                                      name=f"rcp{p}{h2}{nh}")
                        nc.vector.reciprocal(rc[:], O[h2][64:65, :])
                        Rb = psS.tile([64, 512], F32, tag="s",
                                      name=f"rbp{p}{h2}{nh}")
                        nc.tensor.matmul(Rb[:], ones_r[:], rc[:],
                                         start=True, stop=True)
                        rbs = prb.tile([64, 512], F32, tag="rbs",
                                       name=f"rbsp{p}{h2}{nh}")
                        nc.vector.tensor_copy(rbs[:], Rb[:])
                        nc.vector.tensor_tensor(
                            OnT[p][h2 * 64:(h2 + 1) * 64,
                                   nh * 512:(nh + 1) * 512],
                            O[h2][0:64, :], rbs[:], ALU.mult)

        # ---- Final projection: out = concat_heads(O) @ Wo_slice ----
        with tc.tile_pool(name="pf", bufs=2) as pf, \
             tc.tile_pool(name="psF", bufs=2, space="PSUM") as psF:
            for n8 in range(8):
                for dqh in range(2):
                    po = psF.tile([128, 512], F32, tag="po")
                    for p in range(NP):
                        nc.tensor.matmul(
                            po[:], OnT[p][:, n8 * 128:(n8 + 1) * 128],
                            wo_r[:, p, dqh * 512:(dqh + 1) * 512],
                            start=(p == 0), stop=(p == NP - 1))
                    ob = pf.tile([128, 512], F32, tag="ob")
                    evac(ob[:], po[:])
                    nc.sync.dma_start(
                        out_d[n8 * 128:(n8 + 1) * 128,
                              dqh * 512:(dqh + 1) * 512], ob[:])


def _build():
    nc = bacc.Bacc("TRN2", target_bir_lowering=False, debug=False, num_devices=8)
    x_d = nc.dram_tensor("x", [N, DQ], F32, kind="ExternalInput")
    ctx_d = nc.dram_tensor("ctx", [M, DQ], F32, kind="ExternalInput")
    wq_d = nc.dram_tensor("wq", [DQ, IC], F32, kind="ExternalInput")
    wk_d = nc.dram_tensor("wk", [DQ, IC], F32, kind="ExternalInput")
    wv_d = nc.dram_tensor("wv", [DQ, IC], F32, kind="ExternalInput")
    wo_d = nc.dram_tensor("wo", [IC, DQ], F32, kind="ExternalInput")
    madd_d = nc.dram_tensor("madd", [128, MC], F32, kind="ExternalInput")
    out_d = nc.dram_tensor("out", [N, DQ], F32, kind="ExternalOutput")
    with tile.TileContext(nc) as tc:
        _emit(nc, tc, (x_d, ctx_d, wq_d, wk_d, wv_d, wo_d, madd_d, out_d))
    nc.compile()
    return nc


def kernel(x, context, mask, Wq, Wkv, Wo, bo):
    x = np.asarray(x, dtype=np.float32)
    context = np.asarray(context, dtype=np.float32)
    mask = np.asarray(mask)
    Wq = np.asarray(Wq, dtype=np.float32)
    Wkv = np.asarray(Wkv, dtype=np.float32)
    Wo = np.asarray(Wo, dtype=np.float32)
    bo = np.asarray(bo, dtype=np.float32)

    if "nc" not in _CACHE:
        _CACHE["nc"] = _build()
    nc = _CACHE["nc"]

    in_maps = []
    for c in range(8):
        b, hh = divmod(c, 2)
        cs = hh * IC
        madd = np.where(mask[b], np.float32(0.0), np.float32(-1000.0))
        madd = madd.astype(np.float32).reshape(MC, 128).T
        in_maps.append({
            "x": np.ascontiguousarray(x[b]),
            "ctx": np.ascontiguousarray(context[b]),
            "wq": np.ascontiguousarray(Wq[:, cs:cs + IC]),
            "wk": np.ascontiguousarray(Wkv[:, cs:cs + IC]),
            "wv": np.ascontiguousarray(Wkv[:, DQ + cs:DQ + cs + IC]),
            "wo": np.ascontiguousarray(Wo[cs:cs + IC, :]),
            "madd": np.ascontiguousarray(madd),
        })

    res = run_bass_kernel_spmd(nc, in_maps, core_ids=list(range(8)))
    _CACHE["last_results"] = res

    out = np.empty((B, N, DQ), dtype=np.float32)
    for b in range(B):
        out[b] = res.results[2 * b]["out"] + res.results[2 * b + 1]["out"] \
            + bo[None, :]
    return out
